# revision 11
# baseline (speedup 1.0000x reference)
"""Trainium2 Bass kernel for nn_Disc_edge2 (3-layer dense-graph GNN + MLP head).

Sharding: data-parallel over batch B=16 across 8 cores (2 graphs/core).
Per-graph msg layout: [d=128 partitions, f=16384 free], f = c1*2048 + t*128 + p,
edge (i, j) -> p = i, j = 8t + c1.

v2 design (vs baseline):
- The two per-node broadcast adds (xi over j, xj over i) are fused into ONE
  fp8 DoubleRow matmul: lhsT [k, 2, d] = (xib | xjb) fp8, rhs [k, 2, n] =
  (seli | seljm) fp8 indicators; cost 0.5 cyc/row = half of one bf16 matmul.
- The adjacency mask is injected pre-relu as -57344*(1-A) by a second fp8
  DoubleRow matmul (lhsT group0 = -448 const, rhs group0 = replicated (1-A)).
  relu then kills masked edges, so no DVE mask-multiply is ever needed and
  any engine (ACT/DVE/Pool) can evict with a plain relu.
- Layer-1 evict stores S = e1 + relu(s1') (stt op1=add), so the 0.5 e-blend
  disappears; layer-2 uses (0.5*We2) @ S and agg1 = tree(S) - agg0.
- Evicts are split across ACT (activation relu, accum for L2) and DVE
  (tensor_scalar relu / stt add) by per-layer engine maps; chunks are 1024
  cols to amortize per-op init.
"""

import os
import sys

sys.path.insert(0, "/opt/trn_rl_repo")

import numpy as np

import concourse.bass as bass
from concourse import bacc
import concourse.mybir as mybir
import concourse.tile as tile
from concourse.masks import make_identity
from bass_rust import MatmulPerfMode

F32 = mybir.dt.float32
BF16 = mybir.dt.bfloat16
FP8 = mybir.dt.float8e4
I32 = mybir.dt.int32
AF = mybir.ActivationFunctionType
OP = mybir.AluOpType
DR = MatmulPerfMode.DoubleRow

B, N, DN0, DE0, DH = 16, 128, 64, 16, 128
NCORES = 8
GPC = B // NCORES          # graphs per core
FREE = N * N               # 16384
CH = 1024                  # columns per PSUM chunk
NCH = FREE // CH           # 16 chunks

WEIGHT_NAMES = [
    "w_msg_0", "b_msg_0", "w_node_0", "b_node_0",
    "w_msg_1", "b_msg_1", "w_node_1", "b_node_1",
    "w_msg_2", "b_msg_2", "w_node_2", "b_node_2",
    "w_h1", "b_h1", "w_h2", "b_h2", "w_h3", "b_h3",
]

# evict engine per (layer, chunk): 'A' = ACT, 'V' = DVE, 'P' = Pool-assisted
# (ACT relu to tmp, Pool adds for L1).  Tuned by measurement.
EVICT = {
    0: "AAVAAVAAVAAVAAVA",   # L0: relu -> msg0
    1: "VAVVAVVAVVAVVAVV",   # L1: relu + add msg0 -> msg1 (ACT chunks get DVE add)
    2: "AVAVAVAVAVAVAVAV",   # L2: relu + accum -> hsum
}

_CACHE = {}


def build_nc():
    nc = bacc.Bacc()

    ei_d = nc.declare_dram_parameter("edge_index", [GPC, N, N], I32, isOutput=False)
    x_d = nc.declare_dram_parameter("x", [GPC, N, DN0], F32, isOutput=False)
    ea_d = nc.declare_dram_parameter("edge_attr", [GPC, N, N, DE0], F32, isOutput=False)
    wd = {}
    shapes = {
        "w_msg_0": [2 * DN0 + DE0, DH], "b_msg_0": [DH],
        "w_node_0": [DN0 + DH, DH], "b_node_0": [DH],
        "w_msg_1": [3 * DH, DH], "b_msg_1": [DH],
        "w_node_1": [2 * DH, DH], "b_node_1": [DH],
        "w_msg_2": [3 * DH, DH], "b_msg_2": [DH],
        "w_node_2": [2 * DH, DH], "b_node_2": [DH],
        "w_h1": [DH, DH], "b_h1": [DH],
        "w_h2": [DH, DH], "b_h2": [DH],
        "w_h3": [DH, 1], "b_h3": [1],
    }
    for n_ in WEIGHT_NAMES:
        wd[n_] = nc.declare_dram_parameter(n_, shapes[n_], F32, isOutput=False)
    out_d = nc.declare_dram_parameter("out", [GPC, 1], F32, isOutput=True)

    with tile.TileContext(nc) as tc:
        import contextlib
        stack = contextlib.ExitStack()
        consts = stack.enter_context(tc.tile_pool(name="consts", bufs=1))
        gbuf = stack.enter_context(tc.tile_pool(name="gbuf", bufs=1))
        small = stack.enter_context(tc.tile_pool(name="small", bufs=2))
        zpool = stack.enter_context(tc.tile_pool(name="zp", bufs=3, space="PSUM"))
        spsum = stack.enter_context(tc.tile_pool(name="sp", bufs=2, space="PSUM"))
        dpool = stack.enter_context(tc.tile_pool(name="dp", bufs=1, space="DRAM"))

        # -------- input loads first: head of the sync HWDGE FIFO --------
        e0nat, x0in, aiin = [], [], []
        for g in range(GPC):
            t = gbuf.tile([128, 128], I32, tag="ai")
            nc.sync.dma_start(t[:], ei_d[g])
            aiin.append(t)
            t = gbuf.tile([128, DN0], F32, tag=f"x0_{g}")
            nc.sync.dma_start(t[:], x_d[g])
            x0in.append(t)
        for g in range(GPC):
            t = gbuf.tile([128, N * DE0], F32, tag="e0nat")
            nc.sync.dma_start(t[:], ea_d[g].rearrange("i j k -> i (j k)"))
            e0nat.append(t)

        # -------- constants / weights (scalar-queue DMAs) --------
        def f2b(src_ap, p, name, scale=None):
            tmp = consts.tile([p, 128], F32, tag=f"tmp_{name}")
            nc.scalar.dma_start(tmp[:], src_ap)
            t = consts.tile([p, 128], BF16, tag=name)
            if scale is None:
                nc.vector.tensor_copy(t[:], tmp[:])
            else:
                nc.vector.tensor_scalar_mul(t[:], tmp[:], scale)
            return t

        ident = consts.tile([128, 128], F32, tag="ident")
        make_identity(nc, ident[:])

        w = {}
        # 8 block variants of We0: rows [c1*16, c1*16+16) = We0, rest zero
        we0b16 = f2b(wd["w_msg_0"][2 * DN0:, :], DE0, "we0b16")
        we0blk = []
        for c1 in range(8):
            blk = consts.tile([128, DH], BF16, tag=f"we0b{c1}")
            nc.vector.memset(blk[:], 0.0)
            nc.scalar.dma_start(blk[c1 * 16:(c1 + 1) * 16, :], we0b16[:])
            we0blk.append(blk)

        for l in (1, 2):
            sc = 0.5 if l == 2 else None
            w[f"We{l}"] = f2b(wd[f"w_msg_{l}"][2 * DH:, :], DH, f"We{l}", sc)
        w["Wx0"] = f2b(wd["w_node_0"][0:DN0, :], DN0, "Wx0")
        w["Wa0"] = f2b(wd["w_node_0"][DN0:, :], DH, "Wa0")
        w["Wx1"] = f2b(wd["w_node_1"][0:DH, :], DH, "Wx1")
        w["Wa1"] = f2b(wd["w_node_1"][DH:, :], DH, "Wa1")
        # Wi/Wj stay f32->bf16 for the xi'/xj' projections (layer-2 halved)
        w["Wi0"] = f2b(wd["w_msg_0"][0:DN0, :], DN0, "Wi0")
        w["Wj0"] = f2b(wd["w_msg_0"][DN0:2 * DN0, :], DN0, "Wj0")
        for l in (1, 2):
            sc = 0.5 if l == 2 else None
            w[f"Wi{l}"] = f2b(wd[f"w_msg_{l}"][0:DH, :], DH, f"Wi{l}", sc)
            w[f"Wj{l}"] = f2b(wd[f"w_msg_{l}"][DH:2 * DH, :], DH, f"Wj{l}", sc)
        # head weights stay f32
        wh1 = consts.tile([DH, DH], F32, tag="wh1")
        nc.scalar.dma_start(wh1[:], wd["w_h1"][:, :])
        wh2 = consts.tile([DH, DH], F32, tag="wh2")
        nc.scalar.dma_start(wh2[:], wd["w_h2"][:, :])
        wh3 = consts.tile([DH, 1], F32, tag="wh3")
        nc.scalar.dma_start(wh3[:], wd["w_h3"][:, :])

        # bias rows replicated across partitions via 0-step broadcast DMA
        brep = {}
        for l in range(3):
            rep = consts.tile([128, DH], F32, tag=f"brep{l}")
            nc.scalar.dma_start(
                rep[:], wd[f"b_msg_{l}"][:].unsqueeze(0).to_broadcast([128, DH]))
            brep[l] = rep
        bcol = {}
        for nm in ("b_node_0", "b_node_1", "b_h1", "b_h2"):
            c = consts.tile([DH, 1], F32, tag=f"col_{nm}")
            nc.scalar.dma_start(c[:], wd[nm][:].unsqueeze(1))
            bcol[nm] = c
        bh3 = consts.tile([1, 1], F32, tag="col_bh3")
        nc.scalar.dma_start(bh3[:], wd["b_h3"][:].unsqueeze(1))

        # ---- SELIJ fp8 DoubleRow indicator buffer ----
        # [128, c1:8, th:2, r:2, u:8, p:128]; chunk (c1,th) rhs = [128, 2, 1024]
        # r=0: seli (k == p); r=1: seljm (k == 64*th + 8*u + c1)
        selij = consts.tile([128, 8, 2, 2, 8, 128], FP8, tag="selij")
        nc.gpsimd.memset(selij[:], 0.0)
        nc.gpsimd.affine_select(
            out=selij[:, :, :, 0, :, :], in_=selij[:, :, :, 0, :, :],
            compare_op=OP.not_equal, fill=1.0,
            base=0, pattern=[[0, 8], [0, 2], [0, 8], [-1, 128]],
            channel_multiplier=1)
        nc.gpsimd.affine_select(
            out=selij[:, :, :, 1, :, :], in_=selij[:, :, :, 1, :, :],
            compare_op=OP.not_equal, fill=1.0,
            base=0, pattern=[[-1, 8], [-64, 2], [-8, 8], [0, 128]],
            channel_multiplier=1)

        # bigneg lhsT for the mask matmul: group0 = -448, group1 = 0
        bigneg = consts.tile([128, 2, 128], FP8, tag="bigneg")
        nc.vector.memset(bigneg[:, 0, :], -240.0)
        nc.vector.memset(bigneg[:, 1, :], 0.0)

        # maskneg fp8 [128, c1:8, th:2, r:2, u:8, p:128]; r=0 = (1-A) flat in
        # (c1,t,p) order replicated over partitions, r=1 = 0.  One buffer,
        # group0 re-broadcast per graph.
        maskn = gbuf.tile([128, 8, 2, 2, 8, 128], FP8, tag="maskn")
        nc.gpsimd.memset(maskn[:], 0.0)

        mfd8s = []
        for g in range(GPC):
            # complement mask (1 - A), transpose, reorder to flat (c1,t,p)
            afc = gbuf.tile([128, 128], F32, tag="afc")
            nc.vector.tensor_scalar(afc[:], aiin[g][:], -1.0, 1.0,
                                    OP.mult, OP.add)
            atp = spsum.tile([128, 128], F32, tag="sp")
            nc.tensor.transpose(atp[:], afc[:], ident[:])  # (1-A)T[j,i]
            atb = gbuf.tile([128, 128], FP8, tag="atb")
            nc.scalar.copy(atb[:], atp[:])
            atd = dpool.tile([128, 128], FP8, tag=f"atd{g}")
            nc.scalar.dma_start(atd[:], atb[:])
            mfd = dpool.tile([FREE], FP8, tag=f"mfd{g}")
            nc.scalar.dma_start(
                mfd[:].rearrange("(c t p) -> c t p", c=8, t=16),
                atd[:].rearrange("(t c) p -> c t p", c=8))
            mfd8s.append(mfd)

        def load_maskneg(g):
            src = mfd8s[g][:].rearrange("(a b u p) -> a b u p", a=8, b=2, u=8)
            nc.scalar.dma_start(
                maskn[:, :, :, 0, :, :],
                src.unsqueeze(0).to_broadcast([128, 8, 2, 8, 128]))

        load_maskneg(0)

        # ---------------- per-graph pipeline ----------------
        for g in range(GPC):
            # ---- e0 transpose: E0T[(c1,k16), (t,p)] bf16 ----
            e0t = gbuf.tile([128, N * DE0], BF16, tag="e0t")
            for q in range(2):
                tp = zpool.tile([128, CH], F32, tag="z")
                for r in range(8):
                    t16 = 8 * q + r
                    nc.tensor.transpose(
                        tp[:, r * 128:(r + 1) * 128],
                        e0nat[g][:, 128 * t16:128 * (t16 + 1)], ident[:])
                nc.scalar.copy(e0t[:, q * CH:(q + 1) * CH], tp[:])

            # ---- x0T [c,i] bf16 ----
            x0tp = spsum.tile([128, 128], F32, tag="sp")
            nc.tensor.transpose(x0tp[0:DN0, :], x0in[g][:], ident[:])
            x0T = gbuf.tile([DN0, 128], BF16, tag="x0T")
            nc.scalar.copy(x0T[:], x0tp[0:DN0, :])

            msg0 = gbuf.tile([128, FREE], BF16, tag="msg0")
            msg1 = gbuf.tile([128, FREE], BF16, tag="msg1")
            scratch = gbuf.tile([128, FREE // 4], BF16, tag="scratch")
            etmpA = gbuf.tile([128, CH], BF16, tag="etmpA")
            etmpV = gbuf.tile([128, CH], BF16, tag="etmpV")
            hsumA = gbuf.tile([128, NCH], F32, tag="hsumA")
            hsumV = gbuf.tile([128, NCH], F32, tag="hsumV")
            nc.vector.memset(hsumA[:], 0.0)
            nc.vector.memset(hsumV[:], 0.0)
            agg0T = gbuf.tile([128, 128], BF16, tag="agg0T")

            xT = x0T
            for layer in range(3):
                Wi, Wj = w[f"Wi{layer}"], w[f"Wj{layer}"]
                We = None if layer == 0 else w[f"We{layer}"]
                # xi' = xT.T @ Wi + b ; xj' = xT.T @ Wj  -> fp8 groups
                xibjb = small.tile([128, 2, 128], FP8, tag="xibjb")
                xip = spsum.tile([128, 128], F32, tag="sp")
                nc.tensor.matmul(xip[:], xT[:], Wi[:], start=True, stop=True)
                nc.vector.tensor_add(xibjb[:, 0, :], xip[:], brep[layer][:])
                xjp = spsum.tile([128, 128], F32, tag="sp")
                nc.tensor.matmul(xjp[:], xT[:], Wj[:], start=True, stop=True)
                nc.vector.tensor_copy(xibjb[:, 1, :], xjp[:])

                src_e = msg0 if layer == 1 else (msg1 if layer == 2 else None)
                dst = msg0 if layer == 0 else (msg1 if layer == 1 else None)

                for k in range(NCH):
                    c1, th = divmod(k, 2)
                    cols = slice(k * CH, (k + 1) * CH)
                    z = zpool.tile([128, CH], F32, tag="z")
                    for hf in range(2):  # PSUM-bank-sized halves
                        zh = z[:, hf * 512:(hf + 1) * 512]
                        u4 = slice(hf * 4, (hf + 1) * 4)
                        if layer == 0:
                            nc.tensor.matmul(
                                zh, we0blk[c1][:],
                                e0t[:, th * CH + hf * 512:th * CH + (hf + 1) * 512],
                                start=True, stop=False)
                        else:
                            nc.tensor.matmul(
                                zh, We[:],
                                src_e[:, k * CH + hf * 512:k * CH + (hf + 1) * 512],
                                start=True, stop=False)
                        # mask: z += -57344 * (1-A)  (fp8 DoubleRow, group1 = 0)
                        nc.tensor.matmul(zh, bigneg[:], maskn[:, c1, th, :, u4],
                                         start=False, stop=False, perf_mode=DR)
                        # fused xi+xj broadcast (fp8 DoubleRow)
                        nc.tensor.matmul(zh, xibjb[:], selij[:, c1, th, :, u4],
                                         start=False, stop=True, perf_mode=DR)

                    eng = EVICT[layer][k]
                    if layer == 0:
                        if eng == "A":
                            nc.scalar.activation(dst[:, cols], z[:], AF.Relu)
                        else:
                            nc.vector.tensor_scalar(dst[:, cols], z[:], 0.0,
                                                    None, OP.max)
                    elif layer == 1:
                        if eng == "V":
                            nc.vector.scalar_tensor_tensor(
                                out=dst[:, cols], in0=z[:], scalar=0.0,
                                in1=msg0[:, cols], op0=OP.max, op1=OP.add)
                        else:
                            nc.scalar.activation(etmpA[:], z[:], AF.Relu)
                            nc.vector.tensor_add(dst[:, cols], etmpA[:],
                                                 msg0[:, cols])
                    else:
                        if eng == "A":
                            nc.scalar.activation(etmpA[:], z[:], AF.Relu,
                                                 accum_out=hsumA[:, k:k + 1])
                        else:
                            nc.vector.tensor_scalar(etmpV[:], z[:], 0.0,
                                                    0.0, OP.max, OP.add,
                                                    accum_out=hsumV[:, k:k + 1])

                if layer < 2:
                    # agg tree over (c1,t): 16384 -> 128
                    q4 = FREE // 4
                    src = dst
                    nc.vector.tensor_add(scratch[:, 0:q4], src[:, 0:q4],
                                         src[:, q4:2 * q4])
                    nc.vector.tensor_add(scratch[:, 0:q4], scratch[:, 0:q4],
                                         src[:, 2 * q4:3 * q4])
                    nc.vector.tensor_add(scratch[:, 0:q4], scratch[:, 0:q4],
                                         src[:, 3 * q4:4 * q4])
                    width = q4
                    while width > 128:
                        h = width // 2
                        nc.vector.tensor_add(scratch[:, 0:h], scratch[:, 0:h],
                                             scratch[:, h:width])
                        width = h
                    if layer == 0:
                        nc.vector.tensor_copy(agg0T[:], scratch[:, 0:128])
                        aggT = agg0T
                    else:
                        # agg1 = tree(S) - agg0
                        aggT = small.tile([128, 128], BF16, tag="aggT")
                        nc.vector.tensor_sub(aggT[:], scratch[:, 0:128],
                                             agg0T[:])

                    Wx, Wa = w[f"Wx{layer}"], w[f"Wa{layer}"]
                    xnp = spsum.tile([128, 128], F32, tag="sp")
                    nc.tensor.matmul(xnp[:], Wx[:], xT[:], start=True, stop=False)
                    nc.tensor.matmul(xnp[:], Wa[:], aggT[:], start=False, stop=True)
                    xnT = small.tile([128, 128], BF16, tag="xnT")
                    nc.scalar.activation(xnT[:], xnp[:], AF.Relu,
                                         bias=bcol[f"b_node_{layer}"][:])
                    if layer == 1:
                        # x-residual (x1+x2); the 0.5 is folded into Wi2/Wj2
                        xbl = small.tile([128, 128], BF16, tag="xbl")
                        nc.vector.tensor_add(xbl[:], xnT[:], xT[:])
                        xT = xbl
                    else:
                        xT = xnT

            # re-broadcast maskneg for the next graph as soon as L2 is done
            if g + 1 < GPC:
                load_maskneg(g + 1)

            # ---- readout head ----
            hpre = small.tile([128, 1], F32, tag="hpre")
            hpv = small.tile([128, 1], F32, tag="hpv")
            nc.vector.tensor_reduce(hpre[:], hsumA[:], axis=mybir.AxisListType.X,
                                    op=OP.add)
            nc.vector.tensor_reduce(hpv[:], hsumV[:], axis=mybir.AxisListType.X,
                                    op=OP.add)
            nc.vector.tensor_add(hpre[:], hpre[:], hpv[:])
            h1p = spsum.tile([128, 128], F32, tag="sp")
            nc.tensor.matmul(h1p[:, 0:1], wh1[:], hpre[:], start=True, stop=True)
            h1 = small.tile([128, 1], F32, tag="h1")
            nc.scalar.activation(h1[:], h1p[:, 0:1], AF.Relu,
                                 bias=bcol["b_h1"][:], scale=1.0 / FREE)
            h2p = spsum.tile([128, 128], F32, tag="sp")
            nc.tensor.matmul(h2p[:, 0:1], wh2[:], h1[:], start=True, stop=True)
            h2 = small.tile([128, 1], F32, tag="h2")
            nc.scalar.activation(h2[:], h2p[:, 0:1], AF.Relu, bias=bcol["b_h2"][:])
            h3p = spsum.tile([128, 128], F32, tag="sp")
            nc.tensor.matmul(h3p[0:1, 0:1], wh3[:], h2[:], start=True, stop=True)
            oval = small.tile([1, 1], F32, tag="oval")
            nc.scalar.activation(oval[:], h3p[0:1, 0:1], AF.Identity, bias=bh3[:])
            nc.sync.dma_start(out_d[g:g + 1, :], oval[:])

        stack.close()
    nc.finalize()
    return nc


def kernel(**inputs):
    inputs = {k: np.asarray(v) for k, v in inputs.items()}
    if "nc" not in _CACHE:
        _CACHE["nc"] = build_nc()
    nc = _CACHE["nc"]

    in_maps = []
    for c in range(NCORES):
        m = {
            "edge_index": np.ascontiguousarray(inputs["edge_index"][c * GPC:(c + 1) * GPC]),
            "x": np.ascontiguousarray(inputs["x"][c * GPC:(c + 1) * GPC]),
            "edge_attr": np.ascontiguousarray(inputs["edge_attr"][c * GPC:(c + 1) * GPC]),
        }
        for n_ in WEIGHT_NAMES:
            m[n_] = np.ascontiguousarray(inputs[n_], dtype=np.float32)
        in_maps.append(m)

    from concourse.bass_utils import run_bass_kernel_spmd
    res = run_bass_kernel_spmd(nc, in_maps, list(range(NCORES)))
    out = np.concatenate([np.asarray(res.results[c]["out"]).reshape(-1)
                          for c in range(NCORES)])
    return out.astype(np.float32)


# revision 13
# speedup vs baseline: 1.3898x; 1.3898x over previous
"""Trainium2 Bass kernel for nn_Disc_edge2 (3-layer dense-graph GNN + MLP head).

Sharding: data-parallel over batch B=16 across 8 cores (2 graphs/core).
Per-graph msg layout: [d=128 partitions, f=16384 free], f = c1*2048 + t*128 + p,
edge (i, j) -> p = i, j = 8t + c1.

v3 design:
- All constant / input-derived operands are prepared on the HOST in their
  exact on-chip layouts (fp8/bf16) and DMA'd in: the DoubleRow indicator
  buffer selij, the replicated (1-A) mask buffer, transposed edge_attr,
  transposed x, and every weight block.  No on-chip transposes, memsets or
  gpsimd affine_selects remain.
- Per 512-col PSUM half-chunk: one bf16 e@We matmul + one fp8 DoubleRow
  mask-injection matmul (-240*128*(1-A) pre-relu) + one fp8 DoubleRow
  fused xi+xj broadcast matmul.
- Layer-1 evict stores S = e1 + relu(s1') (stt op1=add), so the 0.5 e-blend
  disappears; layer-2 uses (0.5*We2) @ S and agg1 = tree(S) - agg0.
- Evicts split across ACT / DVE by per-layer engine maps.
"""

import sys

sys.path.insert(0, "/opt/trn_rl_repo")

import numpy as np
import ml_dtypes

import concourse.bass as bass
from concourse import bacc
import concourse.mybir as mybir
import concourse.tile as tile
from bass_rust import MatmulPerfMode

F32 = mybir.dt.float32
BF16 = mybir.dt.bfloat16
FP8 = mybir.dt.float8e4
AF = mybir.ActivationFunctionType
OP = mybir.AluOpType
DR = MatmulPerfMode.DoubleRow

NPBF = ml_dtypes.bfloat16
NPF8 = ml_dtypes.float8_e4m3

B, N, DN0, DE0, DH = 16, 128, 64, 16, 128
NCORES = 8
GPC = B // NCORES          # graphs per core
FREE = N * N               # 16384
CH = 1024                  # columns per evict chunk (2 PSUM banks)
NCH = FREE // CH           # 16 chunks

WEIGHT_NAMES = [
    "w_msg_0", "b_msg_0", "w_node_0", "b_node_0",
    "w_msg_1", "b_msg_1", "w_node_1", "b_node_1",
    "w_msg_2", "b_msg_2", "w_node_2", "b_node_2",
    "w_h1", "b_h1", "w_h2", "b_h2", "w_h3", "b_h3",
]

# evict engine per (layer, chunk): 'A' = ACT, 'V' = DVE
EVICT = {
    0: "AAAVAAAVAAAVAAAV",   # L0: relu -> msg0
    1: "VAVVVAVVVAVVVAVV",   # L1: relu + add msg0 -> msg1 (ACT chunks: DVE add)
    2: "AAAVAAVAAAVAAAVA",   # L2: relu + accum -> hsum
}

_CACHE = {}


def _prep_const():
    """Input-independent operand blocks (shared by all cores)."""
    kk = np.arange(128)
    # selij [128, c1:8, th:2, r:2, u:8, p:128]
    sel = np.zeros((128, 8, 2, 2, 8, 128), np.float32)
    eye = (kk[:, None] == kk[None, :]).astype(np.float32)
    sel[:, :, :, 0, :, :] = eye[:, None, None, None, :]
    c1g, thg, ug = np.meshgrid(np.arange(8), np.arange(2), np.arange(8),
                               indexing="ij")
    jmap = 64 * thg + 8 * ug + c1g                      # [8,2,8]
    sel[:, :, :, 1, :, :] = (kk[:, None, None, None, None]
                             == jmap[None, :, :, :, None]).astype(np.float32)
    bigneg = np.zeros((128, 2, 128), np.float32)
    bigneg[:, 0, :] = -240.0
    return sel.astype(NPF8), bigneg.astype(NPF8)


def _prep_weights(inputs):
    f = {k: np.asarray(inputs[k], np.float32) for k in WEIGHT_NAMES}
    o = {}
    wm0 = f["w_msg_0"]
    o["wi0"] = wm0[0:DN0].astype(NPBF)
    o["wj0"] = wm0[DN0:2 * DN0].astype(NPBF)
    we0 = wm0[2 * DN0:]                                  # [16, 128]
    blk = np.zeros((8, 128, DH), np.float32)
    for c1 in range(8):
        blk[c1, c1 * 16:(c1 + 1) * 16] = we0
    o["we0blk"] = blk.astype(NPBF)
    for l, sc in ((1, 1.0), (2, 0.5)):
        wm = f[f"w_msg_{l}"]
        o[f"wi{l}"] = (sc * wm[0:DH]).astype(NPBF)
        o[f"wj{l}"] = (sc * wm[DH:2 * DH]).astype(NPBF)
        o[f"we{l}"] = (sc * wm[2 * DH:]).astype(NPBF)
    for l in (0, 1):
        wn = f[f"w_node_{l}"]
        dn = DN0 if l == 0 else DH
        o[f"wx{l}"] = wn[0:dn].astype(NPBF)
        o[f"wa{l}"] = wn[dn:].astype(NPBF)
        o[f"bnode{l}"] = f[f"b_node_{l}"].reshape(DH, 1)
    for l in range(3):
        o[f"brep{l}"] = np.broadcast_to(f[f"b_msg_{l}"], (128, DH)).copy()
    o["wh1"], o["wh2"], o["wh3"] = f["w_h1"], f["w_h2"], f["w_h3"]
    o["bh1"] = f["b_h1"].reshape(DH, 1)
    o["bh2"] = f["b_h2"].reshape(DH, 1)
    o["bh3"] = f["b_h3"].reshape(1, 1)
    return o


def _prep_graphs(ei, x, ea):
    """Per-core input-derived tensors. ei [GPC,N,N], x [GPC,N,DN0],
    ea [GPC,N,N,DE0] -> maskn fp8, e0T bf16, x0T bf16."""
    tt = np.arange(16)
    # maskn[g, k, c1, th, r=0, u, p] = 1 - A[p, 64*th+8*u+c1]
    A = ei.astype(np.float32)                            # [g, i, j]
    c1g, thg, ug = np.meshgrid(np.arange(8), np.arange(2), np.arange(8),
                               indexing="ij")
    jmap = 64 * thg + 8 * ug + c1g                       # [8,2,8]
    m = 1.0 - A[:, :, jmap]                              # [g, p, 8,2,8]
    m = np.transpose(m, (0, 2, 3, 4, 1))                 # [g, 8,2,8, p]
    maskn = np.zeros((GPC, 128, 8, 2, 2, 8, 128), np.float32)
    maskn[:, :, :, :, 0, :, :] = m[:, None]              # replicate over k
    # e0T[g, c1*16+k16, t*128+p] = ea[g, p, 8t+c1, k16]
    e = np.transpose(ea, (0, 3, 2, 1))                   # [g, k16, j, p]
    e = e.reshape(GPC, DE0, 16, 8, 128)                  # j = 8t+c1 -> [t, c1]
    e = np.transpose(e, (0, 3, 1, 2, 4))                 # [g, c1, k16, t, p]
    e0T = e.reshape(GPC, 128, 16 * 128)
    x0T = np.transpose(x, (0, 2, 1))                     # [g, DN0, N]
    return maskn.astype(NPF8), e0T.astype(NPBF), x0T.astype(NPBF)


def make_core_inputs(inputs, c):
    inputs = {k: np.asarray(v) for k, v in inputs.items()}
    if "_const" not in _CACHE:
        _CACHE["_const"] = _prep_const()
    sel, bigneg = _CACHE["_const"]
    w = _prep_weights(inputs)
    s = slice(c * GPC, (c + 1) * GPC)
    maskn, e0T, x0T = _prep_graphs(
        inputs["edge_index"][s], inputs["x"][s], inputs["edge_attr"][s])
    m = {"selij": sel, "bigneg": bigneg, "maskn": maskn,
         "e0T": e0T, "x0T": x0T}
    m.update(w)
    return m


def build_nc():
    nc = bacc.Bacc()

    P = {}
    decls = {
        "selij": ([128, 8, 2, 2, 8, 128], FP8),
        "bigneg": ([128, 2, 128], FP8),
        "maskn": ([GPC, 128, 8, 2, 2, 8, 128], FP8),
        "e0T": ([GPC, 128, 16 * 128], BF16),
        "x0T": ([GPC, DN0, N], BF16),
        "wi0": ([DN0, DH], BF16), "wj0": ([DN0, DH], BF16),
        "we0blk": ([8, 128, DH], BF16),
        "wi1": ([DH, DH], BF16), "wj1": ([DH, DH], BF16),
        "we1": ([DH, DH], BF16),
        "wi2": ([DH, DH], BF16), "wj2": ([DH, DH], BF16),
        "we2": ([DH, DH], BF16),
        "wx0": ([DN0, DH], BF16), "wa0": ([DH, DH], BF16),
        "wx1": ([DH, DH], BF16), "wa1": ([DH, DH], BF16),
        "bnode0": ([DH, 1], F32), "bnode1": ([DH, 1], F32),
        "brep0": ([128, DH], F32), "brep1": ([128, DH], F32),
        "brep2": ([128, DH], F32),
        "wh1": ([DH, DH], F32), "wh2": ([DH, DH], F32), "wh3": ([DH, 1], F32),
        "bh1": ([DH, 1], F32), "bh2": ([DH, 1], F32), "bh3": ([1, 1], F32),
    }
    for name, (shape, dt) in decls.items():
        P[name] = nc.declare_dram_parameter(name, shape, dt, isOutput=False)
    out_d = nc.declare_dram_parameter("out", [GPC, 1], F32, isOutput=True)

    with tile.TileContext(nc) as tc:
        import contextlib
        stack = contextlib.ExitStack()
        consts = stack.enter_context(tc.tile_pool(name="consts", bufs=1))
        gbuf = stack.enter_context(tc.tile_pool(name="gbuf", bufs=1))
        small = stack.enter_context(tc.tile_pool(name="small", bufs=2))
        zpool = stack.enter_context(tc.tile_pool(name="zp", bufs=3, space="PSUM"))
        spsum = stack.enter_context(tc.tile_pool(name="sp", bufs=2, space="PSUM"))

        # -------- big input-derived loads first (sync HWDGE queue) --------
        selij = consts.tile([128, 8, 2, 2, 8, 128], FP8, tag="selij")
        nc.sync.dma_start(selij[:], P["selij"][:])
        maskns, e0Ts, x0Ts = [], [], []
        for g in range(GPC):
            mk = gbuf.tile([128, 8, 2, 2, 8, 128], FP8, tag=f"maskn{g}")
            nc.sync.dma_start(mk[:], P["maskn"][g])
            maskns.append(mk)
            et = gbuf.tile([128, 16 * 128], BF16, tag=f"e0T{g}")
            nc.sync.dma_start(et[:], P["e0T"][g])
            e0Ts.append(et)
            xt = gbuf.tile([DN0, N], BF16, tag=f"x0T{g}")
            nc.sync.dma_start(xt[:], P["x0T"][g])
            x0Ts.append(xt)

        # -------- weights / consts (scalar queue) --------
        cw = {}
        for name, (shape, dt) in decls.items():
            if name in ("selij", "maskn", "e0T", "x0T", "bigneg", "we0blk"):
                continue
            t = consts.tile(list(shape), dt, tag=f"c_{name}")
            nc.scalar.dma_start(t[:], P[name][:])
            cw[name] = t
        bigneg = consts.tile([128, 2, 128], FP8, tag="bigneg")
        nc.scalar.dma_start(bigneg[:], P["bigneg"][:])
        we0blk = []
        for c1 in range(8):
            t = consts.tile([128, DH], BF16, tag=f"we0b{c1}")
            nc.scalar.dma_start(t[:], P["we0blk"][c1])
            we0blk.append(t)

        # ---------------- per-graph pipeline ----------------
        for g in range(GPC):
            maskn = maskns[g]
            msg0 = gbuf.tile([128, FREE], BF16, tag="msg0")
            msg1 = gbuf.tile([128, FREE], BF16, tag="msg1")
            scratch = gbuf.tile([128, FREE // 4], BF16, tag="scratch")
            etmpA = gbuf.tile([128, CH], BF16, tag="etmpA")
            etmpV = gbuf.tile([128, CH], BF16, tag="etmpV")
            hsumA = gbuf.tile([128, NCH], F32, tag="hsumA")
            hsumV = gbuf.tile([128, NCH], F32, tag="hsumV")
            nc.vector.memset(hsumA[:], 0.0)
            nc.vector.memset(hsumV[:], 0.0)
            agg0T = gbuf.tile([128, 128], BF16, tag="agg0T")

            xT = x0Ts[g]
            for layer in range(3):
                Wi, Wj = cw[f"wi{layer}"], cw[f"wj{layer}"]
                We = None if layer == 0 else cw[f"we{layer}"]
                # xi' = xT.T @ Wi + b ; xj' = xT.T @ Wj  -> fp8 groups
                xibjb = small.tile([128, 2, 128], FP8, tag="xibjb")
                xip = spsum.tile([128, 128], F32, tag="sp")
                nc.tensor.matmul(xip[:], xT[:], Wi[:], start=True, stop=True)
                nc.vector.tensor_add(xibjb[:, 0, :], xip[:],
                                     cw[f"brep{layer}"][:])
                xjp = spsum.tile([128, 128], F32, tag="sp")
                nc.tensor.matmul(xjp[:], xT[:], Wj[:], start=True, stop=True)
                nc.vector.tensor_copy(xibjb[:, 1, :], xjp[:])

                src_e = msg0 if layer == 1 else (msg1 if layer == 2 else None)
                dst = msg0 if layer == 0 else (msg1 if layer == 1 else None)

                for k in range(NCH):
                    c1, th = divmod(k, 2)
                    cols = slice(k * CH, (k + 1) * CH)
                    z = zpool.tile([128, CH], F32, tag="z")
                    for hf in range(2):  # PSUM-bank-sized halves
                        zh = z[:, hf * 512:(hf + 1) * 512]
                        u4 = slice(hf * 4, (hf + 1) * 4)
                        if layer == 0:
                            nc.tensor.matmul(
                                zh, we0blk[c1][:],
                                e0Ts[g][:, th * CH + hf * 512:
                                        th * CH + (hf + 1) * 512],
                                start=True, stop=False)
                        else:
                            nc.tensor.matmul(
                                zh, We[:],
                                src_e[:, k * CH + hf * 512:
                                      k * CH + (hf + 1) * 512],
                                start=True, stop=False)
                        # mask: z += -240*128*(1-A) (fp8 DoubleRow, group1=0)
                        nc.tensor.matmul(zh, bigneg[:], maskn[:, c1, th, :, u4],
                                         start=False, stop=False, perf_mode=DR)
                        # fused xi+xj broadcast (fp8 DoubleRow)
                        nc.tensor.matmul(zh, xibjb[:], selij[:, c1, th, :, u4],
                                         start=False, stop=True, perf_mode=DR)

                    eng = EVICT[layer][k]
                    if layer == 0:
                        if eng == "A":
                            nc.scalar.activation(dst[:, cols], z[:], AF.Relu)
                        else:
                            nc.vector.tensor_scalar(dst[:, cols], z[:], 0.0,
                                                    None, OP.max)
                    elif layer == 1:
                        if eng == "V":
                            nc.vector.scalar_tensor_tensor(
                                out=dst[:, cols], in0=z[:], scalar=0.0,
                                in1=msg0[:, cols], op0=OP.max, op1=OP.add)
                        else:
                            nc.scalar.activation(etmpA[:], z[:], AF.Relu)
                            nc.vector.tensor_add(dst[:, cols], etmpA[:],
                                                 msg0[:, cols])
                    else:
                        if eng == "A":
                            nc.scalar.activation(etmpA[:], z[:], AF.Relu,
                                                 accum_out=hsumA[:, k:k + 1])
                        else:
                            nc.vector.tensor_scalar(etmpV[:], z[:], 0.0,
                                                    0.0, OP.max, OP.add,
                                                    accum_out=hsumV[:, k:k + 1])

                if layer < 2:
                    # agg tree over (c1,t): 16384 -> 128
                    q4 = FREE // 4
                    src = dst
                    nc.vector.tensor_add(scratch[:, 0:q4], src[:, 0:q4],
                                         src[:, q4:2 * q4])
                    nc.vector.tensor_add(scratch[:, 0:q4], scratch[:, 0:q4],
                                         src[:, 2 * q4:3 * q4])
                    nc.vector.tensor_add(scratch[:, 0:q4], scratch[:, 0:q4],
                                         src[:, 3 * q4:4 * q4])
                    width = q4
                    while width > 128:
                        h = width // 2
                        nc.vector.tensor_add(scratch[:, 0:h], scratch[:, 0:h],
                                             scratch[:, h:width])
                        width = h
                    if layer == 0:
                        nc.vector.tensor_copy(agg0T[:], scratch[:, 0:128])
                        aggT = agg0T
                    else:
                        # agg1 = tree(S) - agg0
                        aggT = small.tile([128, 128], BF16, tag="aggT")
                        nc.vector.tensor_sub(aggT[:], scratch[:, 0:128],
                                             agg0T[:])

                    Wx, Wa = cw[f"wx{layer}"], cw[f"wa{layer}"]
                    xnp = spsum.tile([128, 128], F32, tag="sp")
                    nc.tensor.matmul(xnp[:], Wx[:], xT[:], start=True, stop=False)
                    nc.tensor.matmul(xnp[:], Wa[:], aggT[:], start=False, stop=True)
                    xnT = small.tile([128, 128], BF16, tag="xnT")
                    nc.scalar.activation(xnT[:], xnp[:], AF.Relu,
                                         bias=cw[f"bnode{layer}"][:])
                    if layer == 1:
                        # x-residual (x1+x2); the 0.5 is folded into Wi2/Wj2
                        xbl = small.tile([128, 128], BF16, tag="xbl")
                        nc.vector.tensor_add(xbl[:], xnT[:], xT[:])
                        xT = xbl
                    else:
                        xT = xnT

            # ---- readout head ----
            hpre = small.tile([128, 1], F32, tag="hpre")
            hpv = small.tile([128, 1], F32, tag="hpv")
            nc.vector.tensor_reduce(hpre[:], hsumA[:], axis=mybir.AxisListType.X,
                                    op=OP.add)
            nc.vector.tensor_reduce(hpv[:], hsumV[:], axis=mybir.AxisListType.X,
                                    op=OP.add)
            nc.vector.tensor_add(hpre[:], hpre[:], hpv[:])
            h1p = spsum.tile([128, 128], F32, tag="sp")
            nc.tensor.matmul(h1p[:, 0:1], cw["wh1"][:], hpre[:],
                             start=True, stop=True)
            h1 = small.tile([128, 1], F32, tag="h1")
            nc.scalar.activation(h1[:], h1p[:, 0:1], AF.Relu,
                                 bias=cw["bh1"][:], scale=1.0 / FREE)
            h2p = spsum.tile([128, 128], F32, tag="sp")
            nc.tensor.matmul(h2p[:, 0:1], cw["wh2"][:], h1[:],
                             start=True, stop=True)
            h2 = small.tile([128, 1], F32, tag="h2")
            nc.scalar.activation(h2[:], h2p[:, 0:1], AF.Relu, bias=cw["bh2"][:])
            h3p = spsum.tile([128, 128], F32, tag="sp")
            nc.tensor.matmul(h3p[0:1, 0:1], cw["wh3"][:], h2[:],
                             start=True, stop=True)
            oval = small.tile([1, 1], F32, tag="oval")
            nc.scalar.activation(oval[:], h3p[0:1, 0:1], AF.Identity,
                                 bias=cw["bh3"][:])
            nc.sync.dma_start(out_d[g:g + 1, :], oval[:])

        stack.close()
    nc.finalize()
    return nc


def kernel(**inputs):
    inputs = {k: np.asarray(v) for k, v in inputs.items()}
    if "nc" not in _CACHE:
        _CACHE["nc"] = build_nc()
    nc = _CACHE["nc"]

    in_maps = [make_core_inputs(inputs, c) for c in range(NCORES)]

    from concourse.bass_utils import run_bass_kernel_spmd
    res = run_bass_kernel_spmd(nc, in_maps, list(range(NCORES)))
    out = np.concatenate([np.asarray(res.results[c]["out"]).reshape(-1)
                          for c in range(NCORES)])
    return out.astype(np.float32)
